# revision 5
# baseline (speedup 1.0000x reference)
"""Trainium2 Bass kernel for nn_Attention_65420941853381.

MHA with interleaved-sinusoidal positional encodings added to q/k, fused QKV
projections, key-padding + causal masking, softmax, and output projection.

Sharding: 8 cores = 2 batches x 4 head-groups (4 heads each). Each core
computes its 4 heads' attention for one batch plus its partial output
projection; partials are summed on the host.

Device layout (per core, b = core//4, head-group hp = core%4):
  - Projections produce q/k head-dims TRANSPOSED ([head-dim, token]) so the
    scores matmul needs no on-device transposes, and scores come out as
    [key, query] blocks so the key-padding mask is a per-partition bias of
    the exp() activation (ACT fuses: exp(scores + bias)).
  - Softmax runs without max-subtraction: weights are scale 0.02 so scores
    are O(5); masked entries get -1e7 and exp underflows to exactly 0.
    The denominator comes free as a 65th "ones" column in the V slab.
  - Causal masking skips fully-masked score blocks entirely (~37% of the
    score/AV matmul work) and adds a single [128,128] -1e7 triangle to the
    diagonal blocks.
  - Rows whose keys are ALL masked (prefix of padded keys) are degenerate
    (0/0 in the no-max-sub scheme); they are recomputed exactly on host.
"""

import sys

if "/opt/trn_rl_repo" not in sys.path:
    sys.path.insert(0, "/opt/trn_rl_repo")

import numpy as np

import concourse.bass as bass
import concourse.mybir as mybir
import concourse.tile as tile
from concourse import bacc
from concourse.bass_utils import run_bass_kernel_spmd
from concourse.masks import make_identity

B, L, D, H = 2, 2048, 1024, 16
DH = D // H            # 64
NEG = 10000000.0
N_CORES = 8
HPC = H // (N_CORES // B)   # heads per core = 4
CPD = 256                   # output cols per core = HPC * DH

F32 = mybir.dt.float32
AF = mybir.ActivationFunctionType
ADD = mybir.AluOpType.add

_PROGRAM_CACHE = {}


def _build_program():
    nc = bacc.Bacc("TRN2", target_bir_lowering=False, debug=False,
                   num_devices=N_CORES)

    xq_d = nc.dram_tensor("xq", [D, L], F32, kind="ExternalInput")
    xk_d = nc.dram_tensor("xk", [D, L], F32, kind="ExternalInput")
    xv_d = nc.dram_tensor("xv", [D, L], F32, kind="ExternalInput")
    wq_d = nc.dram_tensor("wq", [D, CPD], F32, kind="ExternalInput")
    wk_d = nc.dram_tensor("wk", [D, CPD], F32, kind="ExternalInput")
    wv_d = nc.dram_tensor("wv", [D, CPD], F32, kind="ExternalInput")
    wo_d = nc.dram_tensor("wo", [CPD, D], F32, kind="ExternalInput")
    bq_d = nc.dram_tensor("bq2", [128, 2], F32, kind="ExternalInput")
    bk_d = nc.dram_tensor("bk2", [128, 2], F32, kind="ExternalInput")
    km_d = nc.dram_tensor("kmask", [128, L // 128], F32, kind="ExternalInput")
    cm_d = nc.dram_tensor("cmask", [128, 128], F32, kind="ExternalInput")
    y_d = nc.dram_tensor("y", [L, D], F32, kind="ExternalOutput")

    NT = L // 128   # 16 token tiles
    NB = L // 512   # 4 token blocks

    with tile.TileContext(nc) as tc:
        with tc.tile_pool(name="slab", bufs=1) as slab, \
             tc.tile_pool(name="consts", bufs=1) as consts:
            qa = slab.tile([128, 2, L], F32, tag="qa")     # [pair-dims, chunk, token]
            ka = slab.tile([128, 2, L], F32, tag="ka")
            vp = slab.tile([128, NT, HPC * (DH + 1)], F32, tag="vp")
            yt = slab.tile([128, 2, L], F32, tag="yt")
            attn = slab.tile([128, NT, 512], F32, tag="attn")

            ident = consts.tile([128, 128], F32, tag="ident")
            make_identity(nc, ident[:])
            km_sb = consts.tile([128, NT], F32, tag="km")
            cm_sb = consts.tile([128, 128], F32, tag="cm")
            bq_sb = consts.tile([128, 2], F32, tag="bq")
            bk_sb = consts.tile([128, 2], F32, tag="bk")
            nc.sync.dma_start(km_sb[:], km_d.ap())
            nc.sync.dma_start(cm_sb[:], cm_d.ap())
            nc.sync.dma_start(bq_sb[:], bq_d.ap())
            nc.sync.dma_start(bk_sb[:], bk_d.ap())

            # ones columns of the V slab (softmax denominator trick)
            for e in range(HPC):
                nc.vector.memset(vp[:, :, e * 65 + 64: e * 65 + 65], 1.0)

            # ---------------- Phase A: QKV projections ----------------
            with tc.tile_pool(name="wsl", bufs=1) as wsl, \
                 tc.tile_pool(name="xp", bufs=3) as xp, \
                 tc.tile_pool(name="psA", bufs=2, space="PSUM") as psA, \
                 tc.tile_pool(name="psV", bufs=2, space="PSUM") as psV:
                wq_sb = wsl.tile([128, 8, CPD], F32, tag="wq")
                wk_sb = wsl.tile([128, 8, CPD], F32, tag="wk")
                wv_sb = wsl.tile([128, 8, CPD], F32, tag="wv")
                for wd, wt in ((wq_d, wq_sb), (wk_d, wk_sb), (wv_d, wv_sb)):
                    nc.sync.dma_start(
                        wt[:], wd.ap().rearrange("(c p) n -> p c n", p=128))

                for tb in range(NB):
                    ts = slice(tb * 512, (tb + 1) * 512)
                    xq_t = xp.tile([128, 8, 512], F32, tag="x")
                    xk_t = xp.tile([128, 8, 512], F32, tag="x")
                    xv_t = xp.tile([128, 8, 512], F32, tag="x")
                    for xd, xt in ((xq_d, xq_t), (xk_d, xk_t), (xv_d, xv_t)):
                        nc.sync.dma_start(
                            xt[:],
                            xd.ap().rearrange("(c p) t -> p c t", p=128)[:, :, ts])
                    # Q/K projections, transposed out: [dout-pair, token]
                    for (w_sb, b_sb, acc) in ((wq_sb, bq_sb, qa),
                                              (wk_sb, bk_sb, ka)):
                        for m in range(2):
                            pq = psA.tile([128, 512], F32, tag="pq")
                            src = xq_t if acc is qa else xk_t
                            for ci in range(8):
                                nc.tensor.matmul(
                                    pq[:],
                                    w_sb[:, ci, m * 128:(m + 1) * 128],
                                    src[:, ci, :],
                                    start=(ci == 0), stop=(ci == 7))
                            nc.scalar.activation(acc[:, m, ts], pq[:],
                                                 AF.Identity,
                                                 bias=b_sb[:, m:m + 1])
                    # V projection, natural out: [token, dout]
                    for t4 in range(4):
                        tt = tb * 4 + t4
                        pv = psV.tile([128, CPD], F32, tag="pv")
                        for ci in range(8):
                            nc.tensor.matmul(
                                pv[:],
                                xv_t[:, ci, t4 * 128:(t4 + 1) * 128],
                                wv_sb[:, ci, :],
                                start=(ci == 0), stop=(ci == 7))
                        for e in range(HPC):
                            nc.scalar.copy(vp[:, tt, e * 65: e * 65 + 64],
                                           pv[:, e * 64:(e + 1) * 64])

            # ---------------- Phase B: attention ----------------
            with tc.tile_pool(name="ohp", bufs=6) as ohp, \
                 tc.tile_pool(name="rp", bufs=4) as rp, \
                 tc.tile_pool(name="psS", bufs=3, space="PSUM") as psS, \
                 tc.tile_pool(name="psAV", bufs=3, space="PSUM") as psAV, \
                 tc.tile_pool(name="psT", bufs=2, space="PSUM") as psT:
                for c in range(2):
                    for qb in range(NB):
                        oh_tiles = [ohp.tile([128, 128], F32, tag="oh",
                                             name=f"oh_{c}_{qb}_{s}")
                                    for s in range(4)]
                        for e in range(2):
                            lh = c * 2 + e
                            prt = slice(e * 64, (e + 1) * 64)
                            for kt in range(4 * qb + 4):
                                r = kt - 4 * qb
                                qlo = 128 * r if r > 0 else 0
                                n = 512 - qlo
                                sp = psS.tile([128, 512], F32, tag="sp")
                                nc.tensor.matmul(
                                    sp[:, 0:n],
                                    ka[prt, c, kt * 128:(kt + 1) * 128],
                                    qa[prt, c, qb * 512 + qlo:(qb + 1) * 512],
                                    start=True, stop=True)
                                if r >= 0:
                                    nc.vector.tensor_tensor(
                                        out=sp[:, 0:128], in0=sp[:, 0:128],
                                        in1=cm_sb[:], op=ADD)
                                nc.scalar.activation(
                                    attn[:, kt, qlo:512], sp[:, 0:n],
                                    AF.Exp, bias=km_sb[:, kt:kt + 1])
                            for s in range(4):
                                qt = 4 * qb + s
                                pav = psAV.tile([128, 65], F32, tag="pav")
                                for kt in range(qt + 1):
                                    nc.tensor.matmul(
                                        pav[:],
                                        attn[:, kt, s * 128:(s + 1) * 128],
                                        vp[:, kt, lh * 65:(lh + 1) * 65],
                                        start=(kt == 0), stop=(kt == qt))
                                rr = rp.tile([128, 1], F32, tag="rr")
                                nc.vector.reciprocal(rr[:], pav[:, 64:65])
                                nc.vector.tensor_scalar_mul(
                                    oh_tiles[s][:, prt], pav[:, 0:64], rr[:])
                        for s in range(4):
                            pt = psT.tile([128, 128], F32, tag="pt")
                            nc.tensor.transpose(pt[:], oh_tiles[s][:], ident[:])
                            tok = (qb * 4 + s) * 128
                            nc.scalar.copy(yt[:, c, tok:tok + 128], pt[:])

            # ---------------- Phase C: output projection ----------------
            with tc.tile_pool(name="wosl", bufs=1) as wosl, \
                 tc.tile_pool(name="yp", bufs=3) as yp, \
                 tc.tile_pool(name="psO", bufs=2, space="PSUM") as psO:
                wo_sb = wosl.tile([128, 2, D], F32, tag="wo")
                nc.sync.dma_start(
                    wo_sb[:], wo_d.ap().rearrange("(c p) n -> p c n", p=128))
                for tt in range(NT):
                    for ob in range(2):
                        po = psO.tile([128, 512], F32, tag="po")
                        for c in range(2):
                            nc.tensor.matmul(
                                po[:],
                                yt[:, c, tt * 128:(tt + 1) * 128],
                                wo_sb[:, c, ob * 512:(ob + 1) * 512],
                                start=(c == 0), stop=(c == 1))
                        yo = yp.tile([128, 512], F32, tag="yo")
                        nc.scalar.copy(yo[:], po[:])
                        nc.sync.dma_start(
                            y_d.ap()[tt * 128:(tt + 1) * 128,
                                     ob * 512:(ob + 1) * 512],
                            yo[:])

    nc.compile()
    return nc


def _pos_encodings():
    half = D // 2
    periods = (1.0 / 10000.0 ** (np.arange(half, dtype=np.float32) / half))
    angles = np.arange(L, dtype=np.float32)[:, None] * periods[None, :]
    pe = np.empty((L, D), dtype=np.float32)
    pe[:, 0::2] = np.sin(angles)
    pe[:, 1::2] = np.cos(angles)
    return pe


def _host_fix_degenerate_rows(y, q, k, v, mask, Wq, bq, Wk, bk, Wv, bv, Wo,
                              bo, pe):
    """Rows q where keys 0..q are all padded are 0/0 on device; recompute
    them exactly (reference semantics: softmax over ALL keys)."""
    scale = DH ** -0.5
    for b in range(B):
        rows = np.nonzero(np.cumprod(mask[b].astype(bool)))[0]
        if len(rows) == 0:
            continue
        kp = (k[b] + pe) @ Wk.T + bk          # [L, D]
        vpj = v[b] @ Wv.T + bv
        kh = kp.reshape(L, H, DH)
        vh = vpj.reshape(L, H, DH)
        for qrow in rows:
            qp = (q[b, qrow] + pe[qrow]) @ Wq.T + bq
            qh = qp.reshape(H, DH)
            m = mask[b] | (np.arange(L) > qrow)          # [L]
            out_h = np.empty((H, DH), np.float32)
            for hh in range(H):
                s = (kh[:, hh, :] @ qh[hh]) * scale - m.astype(np.float32) * NEG
                s = s - s.max()
                w = np.exp(s)
                w /= w.sum()
                out_h[hh] = w @ vh[:, hh, :]
            y[b, qrow] = out_h.reshape(D) @ Wo.T + bo
    return y


def kernel(q, k, v, mask, Wq, bq, Wk, bk, Wv, bv, Wo, bo):
    q, k, v = (np.asarray(a, np.float32) for a in (q, k, v))
    mask = np.asarray(mask)
    Wq, bq, Wk, bk, Wv, bv, Wo, bo = (
        np.asarray(a, np.float32) for a in (Wq, bq, Wk, bk, Wv, bv, Wo, bo))

    if "nc" not in _PROGRAM_CACHE:
        _PROGRAM_CACHE["nc"] = _build_program()
    nc = _PROGRAM_CACHE["nc"]

    pe = _pos_encodings()
    scale = np.float32(DH ** -0.5)

    xq_all = np.ascontiguousarray((q + pe).transpose(0, 2, 1))   # [B, D, L]
    xk_all = np.ascontiguousarray((k + pe).transpose(0, 2, 1))
    xv_all = np.ascontiguousarray(v.transpose(0, 2, 1))
    cmask = np.where(np.arange(128)[:, None] > np.arange(128)[None, :],
                     np.float32(-NEG), np.float32(0.0))

    in_maps = []
    for core in range(N_CORES):
        b, hp = core // (N_CORES // B), core % (N_CORES // B)
        cols = slice(hp * CPD, (hp + 1) * CPD)
        in_maps.append({
            "xq": xq_all[b],
            "xk": xk_all[b],
            "xv": xv_all[b],
            "wq": np.ascontiguousarray((Wq[cols] * scale).T),
            "wk": np.ascontiguousarray(Wk[cols].T),
            "wv": np.ascontiguousarray(Wv[cols].T),
            "wo": np.ascontiguousarray(Wo[:, cols].T),
            "bq2": np.ascontiguousarray((bq[cols] * scale).reshape(2, 128).T),
            "bk2": np.ascontiguousarray(bk[cols].reshape(2, 128).T),
            "kmask": np.ascontiguousarray(
                (-NEG * mask[b].astype(np.float32)).reshape(L // 128, 128).T),
            "cmask": cmask,
        })

    res = run_bass_kernel_spmd(nc, in_maps, list(range(N_CORES)))

    y = np.zeros((B, L, D), np.float32)
    for core in range(N_CORES):
        b = core // (N_CORES // B)
        y[b] += res.results[core]["y"]
    y += bv @ Wo.T + bo
    y = _host_fix_degenerate_rows(y, q, k, v, mask, Wq, bq, Wk, bk, Wv, bv,
                                  Wo, bo, pe)
    return y.astype(np.float32)


# revision 10
# speedup vs baseline: 1.3294x; 1.3294x over previous
"""Trainium2 Bass kernel for nn_Attention_65420941853381.

MHA with interleaved-sinusoidal positional encodings added to q/k, fused QKV
projections, key-padding + causal masking, softmax, and output projection.

Sharding: 8 cores = 2 batches x 4 head-groups (4 heads each). Each core
computes its 4 heads' attention for one batch plus its partial output
projection; partials are summed on the host.

Device layout (per core, b = core//4, head-group hp = core%4):
  - Projections produce q/k head-dims TRANSPOSED ([head-dim, token]) so the
    scores matmul needs no on-device transposes, and scores come out as
    [key, query] blocks so the key-padding mask is a per-partition bias of
    the exp() activation (ACT fuses: exp(scores + bias)).
  - Softmax runs without max-subtraction: weights are scale 0.02 so scores
    are O(5); masked entries get -1e7 and exp underflows to exactly 0.
    The denominator comes free as a 65th "ones" column in the V slab.
  - Causal masking skips fully-masked score blocks entirely (~37% of the
    score/AV matmul work) and adds a single [128,128] -1e7 triangle to the
    diagonal blocks.
  - Rows whose keys are ALL masked (prefix of padded keys) are degenerate
    (0/0 in the no-max-sub scheme); they are recomputed exactly on host.
"""

import sys

if "/opt/trn_rl_repo" not in sys.path:
    sys.path.insert(0, "/opt/trn_rl_repo")

import numpy as np

import concourse.bass as bass
import concourse.mybir as mybir
import concourse.tile as tile
from concourse import bacc
from concourse.bass_utils import run_bass_kernel_spmd
from concourse.masks import make_identity

B, L, D, H = 2, 2048, 1024, 16
DH = D // H            # 64
NEG = 10000000.0
N_CORES = 8
HPC = H // (N_CORES // B)   # heads per core = 4
CPD = 256                   # output cols per core = HPC * DH

F32 = mybir.dt.float32
AF = mybir.ActivationFunctionType
ADD = mybir.AluOpType.add

_PROGRAM_CACHE = {}


def _build_program():
    nc = bacc.Bacc("TRN2", target_bir_lowering=False, debug=False,
                   num_devices=N_CORES)

    xq_d = nc.dram_tensor("xq", [D, L], F32, kind="ExternalInput")
    xk_d = nc.dram_tensor("xk", [D, L], F32, kind="ExternalInput")
    xv_d = nc.dram_tensor("xv", [D, L], F32, kind="ExternalInput")
    wq_d = nc.dram_tensor("wq", [D, CPD], F32, kind="ExternalInput")
    wk_d = nc.dram_tensor("wk", [D, CPD], F32, kind="ExternalInput")
    wv_d = nc.dram_tensor("wv", [D, CPD], F32, kind="ExternalInput")
    wo_d = nc.dram_tensor("wo", [CPD, D], F32, kind="ExternalInput")
    bq_d = nc.dram_tensor("bq2", [128, 2], F32, kind="ExternalInput")
    bk_d = nc.dram_tensor("bk2", [128, 2], F32, kind="ExternalInput")
    km_d = nc.dram_tensor("kmask", [128, L // 128], F32, kind="ExternalInput")
    cm_d = nc.dram_tensor("cmask", [128, 128], F32, kind="ExternalInput")
    y_d = nc.dram_tensor("y", [L, D], F32, kind="ExternalOutput")

    NT = L // 128   # 16 token tiles
    NB = L // 512   # 4 token blocks

    with tile.TileContext(nc) as tc:
        with tc.tile_pool(name="slab", bufs=1) as slab, \
             tc.tile_pool(name="consts", bufs=1) as consts:
            qa = slab.tile([128, 2, L], F32, tag="qa")     # [pair-dims, chunk, token]
            ka = slab.tile([128, 2, L], F32, tag="ka")
            vp = slab.tile([128, NT, HPC, DH + 1], F32, tag="vp")
            yt = slab.tile([128, 2, L], F32, tag="yt")

            km_sb = consts.tile([128, NT], F32, tag="km")
            cm_sb = consts.tile([128, 128], F32, tag="cm")
            bq_sb = consts.tile([128, 2], F32, tag="bq")
            bk_sb = consts.tile([128, 2], F32, tag="bk")
            nc.sync.dma_start(km_sb[:], km_d.ap())
            nc.sync.dma_start(cm_sb[:], cm_d.ap())
            nc.sync.dma_start(bq_sb[:], bq_d.ap())
            nc.sync.dma_start(bk_sb[:], bk_d.ap())

            # ones columns of the V slab (softmax denominator trick)
            nc.vector.memset(vp[:, :, :, DH:DH + 1], 1.0)

            # ---------------- Phase A: QKV projections ----------------
            with tc.tile_pool(name="wsl", bufs=1) as wsl, \
                 tc.tile_pool(name="xp", bufs=3) as xp, \
                 tc.tile_pool(name="psA", bufs=2, space="PSUM") as psA, \
                 tc.tile_pool(name="psV", bufs=2, space="PSUM") as psV:
                wq_sb = wsl.tile([128, 8, CPD], F32, tag="wq")
                wk_sb = wsl.tile([128, 8, CPD], F32, tag="wk")
                wv_sb = wsl.tile([128, 8, CPD], F32, tag="wv")
                for wd, wt in ((wq_d, wq_sb), (wk_d, wk_sb), (wv_d, wv_sb)):
                    nc.sync.dma_start(
                        wt[:], wd.ap().rearrange("(c p) n -> p c n", p=128))

                for tb in range(NB):
                    ts = slice(tb * 512, (tb + 1) * 512)
                    xq_t = xp.tile([128, 8, 512], F32, tag="x")
                    xk_t = xp.tile([128, 8, 512], F32, tag="x")
                    xv_t = xp.tile([128, 8, 512], F32, tag="x")
                    for xd, xt in ((xq_d, xq_t), (xk_d, xk_t), (xv_d, xv_t)):
                        nc.sync.dma_start(
                            xt[:],
                            xd.ap().rearrange("(c p) t -> p c t", p=128)[:, :, ts])
                    # Q/K projections, transposed out: [dout-pair, token]
                    for (w_sb, b_sb, acc) in ((wq_sb, bq_sb, qa),
                                              (wk_sb, bk_sb, ka)):
                        for m in range(2):
                            pq = psA.tile([128, 512], F32, tag="pq")
                            src = xq_t if acc is qa else xk_t
                            for ci in range(8):
                                nc.tensor.matmul(
                                    pq[:],
                                    w_sb[:, ci, m * 128:(m + 1) * 128],
                                    src[:, ci, :],
                                    start=(ci == 0), stop=(ci == 7))
                            nc.scalar.activation(acc[:, m, ts], pq[:],
                                                 AF.Identity,
                                                 bias=b_sb[:, m:m + 1])
                    # V projection, natural out: [token, dout]
                    for t4 in range(4):
                        tt = tb * 4 + t4
                        pv = psV.tile([128, CPD], F32, tag="pv")
                        for ci in range(8):
                            nc.tensor.matmul(
                                pv[:],
                                xv_t[:, ci, t4 * 128:(t4 + 1) * 128],
                                wv_sb[:, ci, :],
                                start=(ci == 0), stop=(ci == 7))
                        for e in range(HPC):
                            nc.scalar.copy(vp[:, tt, e, 0:DH],
                                           pv[:, e * 64:(e + 1) * 64])

            # ---------------- Phase B: attention ----------------
            # Per (head, 512-query block): interleave
            #   scores [k,q] -> (+causal tri on diag) -> exp(.+kmask bias)
            #   -> AV accumulate: psum[65, 512] = [d(64)+denom(1), q]
            # then divide rows 0..63 by the broadcast denominator row.
            with tc.tile_pool(name="abp", bufs=3) as abp, \
                 tc.tile_pool(name="rp", bufs=3) as rp, \
                 tc.tile_pool(name="rbp", bufs=3) as rbp, \
                 tc.tile_pool(name="psS", bufs=3, space="PSUM") as psS, \
                 tc.tile_pool(name="psAV", bufs=2, space="PSUM") as psAV:
                for c in range(2):
                    for e in range(2):
                        lh = c * 2 + e
                        prt = slice(e * 64, (e + 1) * 64)
                        for qb in range(NB):
                            klast = 4 * qb + 3
                            pav = psAV.tile([65, 512], F32, tag="pav")
                            for kt in range(klast + 1):
                                r = kt - 4 * qb
                                qlo = 128 * r if r > 0 else 0
                                n = 512 - qlo
                                sp = psS.tile([128, 512], F32, tag="sp")
                                nc.tensor.matmul(
                                    sp[:, 0:n],
                                    ka[prt, c, kt * 128:(kt + 1) * 128],
                                    qa[prt, c, qb * 512 + qlo:(qb + 1) * 512],
                                    start=True, stop=True)
                                if r >= 0:
                                    nc.vector.tensor_tensor(
                                        out=sp[:, 0:128], in0=sp[:, 0:128],
                                        in1=cm_sb[:], op=ADD)
                                ab = abp.tile([128, 512], F32, tag="ab")
                                nc.scalar.activation(
                                    ab[:, 0:n], sp[:, 0:n],
                                    AF.Exp, bias=km_sb[:, kt:kt + 1])
                                nc.tensor.matmul(
                                    pav[:, qlo:512],
                                    vp[:, kt, lh, :],
                                    ab[:, 0:n],
                                    start=(kt == 0), stop=(kt == klast))
                            rr = rp.tile([1, 512], F32, tag="rr")
                            nc.vector.reciprocal(rr[:], pav[64:65, :])
                            rb = rbp.tile([64, 512], F32, tag="rb")
                            nc.gpsimd.partition_broadcast(rb[:], rr[:])
                            nc.vector.tensor_tensor(
                                out=yt[prt, c, qb * 512:(qb + 1) * 512],
                                in0=pav[0:64, :], in1=rb[:],
                                op=mybir.AluOpType.mult)

            # ---------------- Phase C: output projection ----------------
            with tc.tile_pool(name="wosl", bufs=1) as wosl, \
                 tc.tile_pool(name="yp", bufs=3) as yp, \
                 tc.tile_pool(name="psO", bufs=2, space="PSUM") as psO:
                wo_sb = wosl.tile([128, 2, D], F32, tag="wo")
                nc.sync.dma_start(
                    wo_sb[:], wo_d.ap().rearrange("(c p) n -> p c n", p=128))
                for tt in range(NT):
                    for ob in range(2):
                        po = psO.tile([128, 512], F32, tag="po")
                        for c in range(2):
                            nc.tensor.matmul(
                                po[:],
                                yt[:, c, tt * 128:(tt + 1) * 128],
                                wo_sb[:, c, ob * 512:(ob + 1) * 512],
                                start=(c == 0), stop=(c == 1))
                        yo = yp.tile([128, 512], F32, tag="yo")
                        nc.scalar.copy(yo[:], po[:])
                        nc.sync.dma_start(
                            y_d.ap()[tt * 128:(tt + 1) * 128,
                                     ob * 512:(ob + 1) * 512],
                            yo[:])

    nc.compile()
    return nc


def _pos_encodings():
    half = D // 2
    periods = (1.0 / 10000.0 ** (np.arange(half, dtype=np.float32) / half))
    angles = np.arange(L, dtype=np.float32)[:, None] * periods[None, :]
    pe = np.empty((L, D), dtype=np.float32)
    pe[:, 0::2] = np.sin(angles)
    pe[:, 1::2] = np.cos(angles)
    return pe


def _host_fix_degenerate_rows(y, q, k, v, mask, Wq, bq, Wk, bk, Wv, bv, Wo,
                              bo, pe):
    """Rows q where keys 0..q are all padded are 0/0 on device; recompute
    them exactly (reference semantics: softmax over ALL keys)."""
    scale = DH ** -0.5
    for b in range(B):
        rows = np.nonzero(np.cumprod(mask[b].astype(bool)))[0]
        if len(rows) == 0:
            continue
        kp = (k[b] + pe) @ Wk.T + bk          # [L, D]
        vpj = v[b] @ Wv.T + bv
        kh = kp.reshape(L, H, DH)
        vh = vpj.reshape(L, H, DH)
        for qrow in rows:
            qp = (q[b, qrow] + pe[qrow]) @ Wq.T + bq
            qh = qp.reshape(H, DH)
            m = mask[b] | (np.arange(L) > qrow)          # [L]
            out_h = np.empty((H, DH), np.float32)
            for hh in range(H):
                s = (kh[:, hh, :] @ qh[hh]) * scale - m.astype(np.float32) * NEG
                s = s - s.max()
                w = np.exp(s)
                w /= w.sum()
                out_h[hh] = w @ vh[:, hh, :]
            y[b, qrow] = out_h.reshape(D) @ Wo.T + bo
    return y


def kernel(q, k, v, mask, Wq, bq, Wk, bk, Wv, bv, Wo, bo):
    q, k, v = (np.asarray(a, np.float32) for a in (q, k, v))
    mask = np.asarray(mask)
    Wq, bq, Wk, bk, Wv, bv, Wo, bo = (
        np.asarray(a, np.float32) for a in (Wq, bq, Wk, bk, Wv, bv, Wo, bo))

    if "nc" not in _PROGRAM_CACHE:
        _PROGRAM_CACHE["nc"] = _build_program()
    nc = _PROGRAM_CACHE["nc"]

    pe = _pos_encodings()
    scale = np.float32(DH ** -0.5)

    xq_all = np.ascontiguousarray((q + pe).transpose(0, 2, 1))   # [B, D, L]
    xk_all = np.ascontiguousarray((k + pe).transpose(0, 2, 1))
    xv_all = np.ascontiguousarray(v.transpose(0, 2, 1))
    cmask = np.where(np.arange(128)[:, None] > np.arange(128)[None, :],
                     np.float32(-NEG), np.float32(0.0))

    in_maps = []
    for core in range(N_CORES):
        b, hp = core // (N_CORES // B), core % (N_CORES // B)
        cols = slice(hp * CPD, (hp + 1) * CPD)
        in_maps.append({
            "xq": xq_all[b],
            "xk": xk_all[b],
            "xv": xv_all[b],
            "wq": np.ascontiguousarray((Wq[cols] * scale).T),
            "wk": np.ascontiguousarray(Wk[cols].T),
            "wv": np.ascontiguousarray(Wv[cols].T),
            "wo": np.ascontiguousarray(Wo[:, cols].T),
            "bq2": np.ascontiguousarray((bq[cols] * scale).reshape(2, 128).T),
            "bk2": np.ascontiguousarray(bk[cols].reshape(2, 128).T),
            "kmask": np.ascontiguousarray(
                (-NEG * mask[b].astype(np.float32)).reshape(L // 128, 128).T),
            "cmask": cmask,
        })

    res = run_bass_kernel_spmd(nc, in_maps, list(range(N_CORES)))

    y = np.zeros((B, L, D), np.float32)
    for core in range(N_CORES):
        b = core // (N_CORES // B)
        y[b] += res.results[core]["y"]
    y += bv @ Wo.T + bo
    y = _host_fix_degenerate_rows(y, q, k, v, mask, Wq, bq, Wk, bk, Wv, bv,
                                  Wo, bo, pe)
    return y.astype(np.float32)


# revision 11
# speedup vs baseline: 1.3743x; 1.0338x over previous
"""Trainium2 Bass kernel for nn_Attention_65420941853381.

MHA with interleaved-sinusoidal positional encodings added to q/k, fused QKV
projections, key-padding + causal masking, softmax, and output projection.

Sharding: 8 cores = 2 batches x 4 head-groups (4 heads each). Each core
computes its 4 heads' attention for one batch plus its partial output
projection; partials are summed on the host.

Device layout (per core, b = core//4, head-group hp = core%4):
  - Projections produce q/k head-dims TRANSPOSED ([head-dim, token]) so the
    scores matmul needs no on-device transposes, and scores come out as
    [key, query] blocks so the key-padding mask is a per-partition bias of
    the exp() activation (ACT fuses: exp(scores + bias)).
  - Softmax runs without max-subtraction: weights are scale 0.02 so scores
    are O(5); masked entries get -1e7 and exp underflows to exactly 0.
    The denominator comes free as a 65th "ones" column in the V slab.
  - Causal masking skips fully-masked score blocks entirely (~37% of the
    score/AV matmul work) and adds a single [128,128] -1e7 triangle to the
    diagonal blocks.
  - Rows whose keys are ALL masked (prefix of padded keys) are degenerate
    (0/0 in the no-max-sub scheme); they are recomputed exactly on host.
"""

import sys

if "/opt/trn_rl_repo" not in sys.path:
    sys.path.insert(0, "/opt/trn_rl_repo")

import numpy as np

import concourse.bass as bass
import concourse.mybir as mybir
import concourse.tile as tile
from concourse import bacc
from concourse.bass_utils import run_bass_kernel_spmd
from concourse.masks import make_identity

B, L, D, H = 2, 2048, 1024, 16
DH = D // H            # 64
NEG = 10000000.0
N_CORES = 8
HPC = H // (N_CORES // B)   # heads per core = 4
CPD = 256                   # output cols per core = HPC * DH

F32 = mybir.dt.float32
AF = mybir.ActivationFunctionType
ADD = mybir.AluOpType.add

_PROGRAM_CACHE = {}


def _build_program():
    nc = bacc.Bacc("TRN2", target_bir_lowering=False, debug=False,
                   num_devices=N_CORES)

    xq_d = nc.dram_tensor("xq", [D, L], F32, kind="ExternalInput")
    xk_d = nc.dram_tensor("xk", [D, L], F32, kind="ExternalInput")
    xv_d = nc.dram_tensor("xv", [D, L], F32, kind="ExternalInput")
    wq_d = nc.dram_tensor("wq", [D, CPD], F32, kind="ExternalInput")
    wk_d = nc.dram_tensor("wk", [D, CPD], F32, kind="ExternalInput")
    wv_d = nc.dram_tensor("wv", [D, CPD], F32, kind="ExternalInput")
    wo_d = nc.dram_tensor("wo", [CPD, D], F32, kind="ExternalInput")
    bq_d = nc.dram_tensor("bq2", [128, 2], F32, kind="ExternalInput")
    bk_d = nc.dram_tensor("bk2", [128, 2], F32, kind="ExternalInput")
    km_d = nc.dram_tensor("kmask", [128, L // 128], F32, kind="ExternalInput")
    cm_d = nc.dram_tensor("cmask", [128, 128], F32, kind="ExternalInput")
    y_d = nc.dram_tensor("y", [L, D], F32, kind="ExternalOutput")

    NT = L // 128   # 16 token tiles
    NB = L // 512   # 4 token blocks

    with tile.TileContext(nc) as tc:
        with tc.tile_pool(name="slab", bufs=1) as slab, \
             tc.tile_pool(name="consts", bufs=1) as consts:
            qa = slab.tile([128, 2, L], F32, tag="qa")     # [pair-dims, chunk, token]
            ka = slab.tile([128, 2, L], F32, tag="ka")
            vp = slab.tile([128, NT, HPC, DH + 1], F32, tag="vp")
            yt = slab.tile([128, 2, L], F32, tag="yt")

            km_sb = consts.tile([128, NT], F32, tag="km")
            cm_sb = consts.tile([128, 128], F32, tag="cm")
            bq_sb = consts.tile([128, 2], F32, tag="bq")
            bk_sb = consts.tile([128, 2], F32, tag="bk")
            nc.sync.dma_start(km_sb[:], km_d.ap())
            nc.sync.dma_start(cm_sb[:], cm_d.ap())
            nc.sync.dma_start(bq_sb[:], bq_d.ap())
            nc.sync.dma_start(bk_sb[:], bk_d.ap())

            # ones columns of the V slab (softmax denominator trick)
            nc.vector.memset(vp[:, :, :, DH:DH + 1], 1.0)

            # ---------------- Phase A: QKV projections ----------------
            with tc.tile_pool(name="wsl", bufs=1) as wsl, \
                 tc.tile_pool(name="xp", bufs=3) as xp, \
                 tc.tile_pool(name="psA", bufs=2, space="PSUM") as psA, \
                 tc.tile_pool(name="psV", bufs=2, space="PSUM") as psV:
                wq_sb = wsl.tile([128, 8, CPD], F32, tag="wq")
                wk_sb = wsl.tile([128, 8, CPD], F32, tag="wk")
                wv_sb = wsl.tile([128, 8, CPD], F32, tag="wv")
                for wd, wt in ((wq_d, wq_sb), (wk_d, wk_sb), (wv_d, wv_sb)):
                    nc.sync.dma_start(
                        wt[:], wd.ap().rearrange("(c p) n -> p c n", p=128))

                for tb in range(NB):
                    ts = slice(tb * 512, (tb + 1) * 512)
                    xq_t = xp.tile([128, 8, 512], F32, tag="x")
                    xk_t = xp.tile([128, 8, 512], F32, tag="x")
                    xv_t = xp.tile([128, 8, 512], F32, tag="x")
                    for xd, xt in ((xq_d, xq_t), (xk_d, xk_t), (xv_d, xv_t)):
                        nc.sync.dma_start(
                            xt[:],
                            xd.ap().rearrange("(c p) t -> p c t", p=128)[:, :, ts])
                    # Q/K projections, transposed out: [dout-pair, token]
                    for (w_sb, b_sb, acc) in ((wq_sb, bq_sb, qa),
                                              (wk_sb, bk_sb, ka)):
                        for m in range(2):
                            pq = psA.tile([128, 512], F32, tag="pq")
                            src = xq_t if acc is qa else xk_t
                            for ci in range(8):
                                nc.tensor.matmul(
                                    pq[:],
                                    w_sb[:, ci, m * 128:(m + 1) * 128],
                                    src[:, ci, :],
                                    start=(ci == 0), stop=(ci == 7))
                            nc.scalar.activation(acc[:, m, ts], pq[:],
                                                 AF.Identity,
                                                 bias=b_sb[:, m:m + 1])
                    # V projection, natural out: [token, dout]
                    for t4 in range(4):
                        tt = tb * 4 + t4
                        pv = psV.tile([128, CPD], F32, tag="pv")
                        for ci in range(8):
                            nc.tensor.matmul(
                                pv[:],
                                xv_t[:, ci, t4 * 128:(t4 + 1) * 128],
                                wv_sb[:, ci, :],
                                start=(ci == 0), stop=(ci == 7))
                        for e in range(HPC):
                            nc.scalar.copy(vp[:, tt, e, 0:DH],
                                           pv[:, e * 64:(e + 1) * 64])

            # ---------------- Phase B: attention ----------------
            # Per (head, 512-query block): interleave
            #   scores [k,q] -> (+causal tri on diag) -> exp(.+kmask bias)
            #   -> AV accumulate: psum[65, 512] = [d(64)+denom(1), q]
            # then divide rows 0..63 by the broadcast denominator row.
            with tc.tile_pool(name="abp", bufs=4) as abp, \
                 tc.tile_pool(name="rp", bufs=4) as rp, \
                 tc.tile_pool(name="rbp", bufs=4) as rbp, \
                 tc.tile_pool(name="psS", bufs=4, space="PSUM") as psS, \
                 tc.tile_pool(name="psAV", bufs=3, space="PSUM") as psAV:
                for c in range(2):
                    for e in range(2):
                        lh = c * 2 + e
                        prt = slice(e * 64, (e + 1) * 64)
                        for qb in range(NB):
                            klast = 4 * qb + 3
                            pav = psAV.tile([65, 512], F32, tag="pav")
                            for kt in range(klast + 1):
                                r = kt - 4 * qb
                                qlo = 128 * r if r > 0 else 0
                                n = 512 - qlo
                                sp = psS.tile([128, 512], F32, tag="sp")
                                nc.tensor.matmul(
                                    sp[:, 0:n],
                                    ka[prt, c, kt * 128:(kt + 1) * 128],
                                    qa[prt, c, qb * 512 + qlo:(qb + 1) * 512],
                                    start=True, stop=True)
                                if r >= 0:
                                    nc.vector.tensor_tensor(
                                        out=sp[:, 0:128], in0=sp[:, 0:128],
                                        in1=cm_sb[:], op=ADD)
                                ab = abp.tile([128, 512], F32, tag="ab")
                                nc.scalar.activation(
                                    ab[:, 0:n], sp[:, 0:n],
                                    AF.Exp, bias=km_sb[:, kt:kt + 1])
                                nc.tensor.matmul(
                                    pav[:, qlo:512],
                                    vp[:, kt, lh, :],
                                    ab[:, 0:n],
                                    start=(kt == 0), stop=(kt == klast))
                            rr = rp.tile([1, 512], F32, tag="rr")
                            nc.vector.reciprocal(rr[:], pav[64:65, :])
                            rb = rbp.tile([64, 512], F32, tag="rb")
                            nc.gpsimd.partition_broadcast(rb[:], rr[:])
                            nc.vector.tensor_tensor(
                                out=yt[prt, c, qb * 512:(qb + 1) * 512],
                                in0=pav[0:64, :], in1=rb[:],
                                op=mybir.AluOpType.mult)

            # ---------------- Phase C: output projection ----------------
            with tc.tile_pool(name="wosl", bufs=1) as wosl, \
                 tc.tile_pool(name="yp", bufs=3) as yp, \
                 tc.tile_pool(name="psO", bufs=2, space="PSUM") as psO:
                wo_sb = wosl.tile([128, 2, D], F32, tag="wo")
                nc.sync.dma_start(
                    wo_sb[:], wo_d.ap().rearrange("(c p) n -> p c n", p=128))
                for tt in range(NT):
                    for ob in range(2):
                        po = psO.tile([128, 512], F32, tag="po")
                        for c in range(2):
                            nc.tensor.matmul(
                                po[:],
                                yt[:, c, tt * 128:(tt + 1) * 128],
                                wo_sb[:, c, ob * 512:(ob + 1) * 512],
                                start=(c == 0), stop=(c == 1))
                        yo = yp.tile([128, 512], F32, tag="yo")
                        nc.scalar.copy(yo[:], po[:])
                        nc.sync.dma_start(
                            y_d.ap()[tt * 128:(tt + 1) * 128,
                                     ob * 512:(ob + 1) * 512],
                            yo[:])

    nc.compile()
    return nc


def _pos_encodings():
    half = D // 2
    periods = (1.0 / 10000.0 ** (np.arange(half, dtype=np.float32) / half))
    angles = np.arange(L, dtype=np.float32)[:, None] * periods[None, :]
    pe = np.empty((L, D), dtype=np.float32)
    pe[:, 0::2] = np.sin(angles)
    pe[:, 1::2] = np.cos(angles)
    return pe


def _host_fix_degenerate_rows(y, q, k, v, mask, Wq, bq, Wk, bk, Wv, bv, Wo,
                              bo, pe):
    """Rows q where keys 0..q are all padded are 0/0 on device; recompute
    them exactly (reference semantics: softmax over ALL keys)."""
    scale = DH ** -0.5
    for b in range(B):
        rows = np.nonzero(np.cumprod(mask[b].astype(bool)))[0]
        if len(rows) == 0:
            continue
        kp = (k[b] + pe) @ Wk.T + bk          # [L, D]
        vpj = v[b] @ Wv.T + bv
        kh = kp.reshape(L, H, DH)
        vh = vpj.reshape(L, H, DH)
        for qrow in rows:
            qp = (q[b, qrow] + pe[qrow]) @ Wq.T + bq
            qh = qp.reshape(H, DH)
            m = mask[b] | (np.arange(L) > qrow)          # [L]
            out_h = np.empty((H, DH), np.float32)
            for hh in range(H):
                s = (kh[:, hh, :] @ qh[hh]) * scale - m.astype(np.float32) * NEG
                s = s - s.max()
                w = np.exp(s)
                w /= w.sum()
                out_h[hh] = w @ vh[:, hh, :]
            y[b, qrow] = out_h.reshape(D) @ Wo.T + bo
    return y


def kernel(q, k, v, mask, Wq, bq, Wk, bk, Wv, bv, Wo, bo):
    q, k, v = (np.asarray(a, np.float32) for a in (q, k, v))
    mask = np.asarray(mask)
    Wq, bq, Wk, bk, Wv, bv, Wo, bo = (
        np.asarray(a, np.float32) for a in (Wq, bq, Wk, bk, Wv, bv, Wo, bo))

    if "nc" not in _PROGRAM_CACHE:
        _PROGRAM_CACHE["nc"] = _build_program()
    nc = _PROGRAM_CACHE["nc"]

    pe = _pos_encodings()
    scale = np.float32(DH ** -0.5)

    xq_all = np.ascontiguousarray((q + pe).transpose(0, 2, 1))   # [B, D, L]
    xk_all = np.ascontiguousarray((k + pe).transpose(0, 2, 1))
    xv_all = np.ascontiguousarray(v.transpose(0, 2, 1))
    cmask = np.where(np.arange(128)[:, None] > np.arange(128)[None, :],
                     np.float32(-NEG), np.float32(0.0))

    in_maps = []
    for core in range(N_CORES):
        b, hp = core // (N_CORES // B), core % (N_CORES // B)
        cols = slice(hp * CPD, (hp + 1) * CPD)
        in_maps.append({
            "xq": xq_all[b],
            "xk": xk_all[b],
            "xv": xv_all[b],
            "wq": np.ascontiguousarray((Wq[cols] * scale).T),
            "wk": np.ascontiguousarray(Wk[cols].T),
            "wv": np.ascontiguousarray(Wv[cols].T),
            "wo": np.ascontiguousarray(Wo[:, cols].T),
            "bq2": np.ascontiguousarray((bq[cols] * scale).reshape(2, 128).T),
            "bk2": np.ascontiguousarray(bk[cols].reshape(2, 128).T),
            "kmask": np.ascontiguousarray(
                (-NEG * mask[b].astype(np.float32)).reshape(L // 128, 128).T),
            "cmask": cmask,
        })

    res = run_bass_kernel_spmd(nc, in_maps, list(range(N_CORES)))

    y = np.zeros((B, L, D), np.float32)
    for core in range(N_CORES):
        b = core // (N_CORES // B)
        y[b] += res.results[core]["y"]
    y += bv @ Wo.T + bo
    y = _host_fix_degenerate_rows(y, q, k, v, mask, Wq, bq, Wk, bk, Wv, bv,
                                  Wo, bo, pe)
    return y.astype(np.float32)
